# revision 23
# baseline (speedup 1.0000x reference)
"""CAN per-sample 2-layer MLP kernel for Trainium2 — P=3 trio variant.

Like the pair kernel, but THREE samples share each matmul:
  * Trio slot: samples at rows {0,17,34} of a 51-row block; two trios per
    128-partition tile at 64-row strides, matmuls at tile_position
    (64j, 64j) so consecutive matmuls alternate row groups and LDWEIGHTS
    pulls ahead (51 cols ~ 21ns @2.4GHz hides under the 50-row stream).
  * L1: 51x51 block-diag Wt0; L2: 51x48 stacked Wt1; out 48 rows/trio.
  * Per core: 2048 samples padded to 2112 = 352 tile-slots (6 samples
    each) = 11 batches x 32 slots; PSUM sub-batch gs=8 slots; 1408
    matmuls/core vs 2048 for pairs.
  * Everything else mirrors kernel.py: fp16 wire, fp32 PSUM, one
    two-bank psum tile per sub-batch, full-width relu1 on ACT / relu2 on
    DVE, SWDGE input DMAs 3 batches ahead, SKEW=2 pipeline, wait
    stripping post-passes.
"""

from contextlib import ExitStack

import numpy as np

import concourse.bass as bass
import concourse.mybir as mybir
from concourse import tile
from concourse.bass_utils import run_bass_kernel_spmd
from concourse.tile_rust import add_dep_helper

def _strip_covered_waits(nc):
    """Remove, from DMACopy instructions, semaphore waits already guaranteed
    by an earlier instruction on the same engine queue. Coverage is killed
    for a sem from the point of any non-increment update (barrier resets)."""
    for fn in nc.m.functions:
        for blk in fn.blocks:
            seen = {}
            for ins in blk.instructions:
                si = ins.sync_info
                if si is None:
                    continue
                eng = ins.engine
                # In-order sequencers: a sem-ge wait satisfied for an earlier
                # instruction on this queue still holds for later ones, so
                # repeats are dead weight. On PE they also fence the 64-deep
                # reorder window (LDWEIGHTS pull-ahead), serializing
                # ldweights with the previous matmul.
                strippable = type(ins).__name__ in ("InstDMACopy", "InstMatmult")
                kept = []
                changed = False
                for w in si.on_wait:
                    if (
                        strippable
                        and w.wait_mode == "sem-ge-imm"
                        and w.wait_reg is None
                        and seen.get((eng, w.id), -1) >= w.wait_value
                    ):
                        changed = True
                        continue
                    kept.append(w)
                for w in kept:
                    if w.wait_mode == "sem-ge-imm" and w.wait_reg is None:
                        key = (eng, w.id)
                        if seen.get(key, -1) < w.wait_value:
                            seen[key] = w.wait_value
                for u in si.on_update:
                    if u.update_mode != "sem-add-imm" or (
                        u.update_value is not None and u.update_value < 0
                    ):
                        for key in [k for k in seen if k[1] == u.id]:
                            del seen[key]
                if changed:
                    ins.sync_info = mybir.SyncInfo(
                        on_wait=kept, on_update=si.on_update
                    )


_WS_COUNT = [0]


def _split_excess_waits(nc, cap=1):
    """Move excess inline waits onto NoOps inserted immediately before, on
    the same engine queue - semantically identical (sequencers execute
    waits in order)."""
    for fn in nc.m.functions:
        for blk in fn.blocks:
            insts = blk.instructions
            i = 0
            while i < len(insts):
                ins = insts[i]
                si = ins.sync_info
                if si is None or len(si.on_wait) <= cap:
                    i += 1
                    continue
                waits = list(si.on_wait)
                keep, extra = waits[-cap:], waits[:-cap]
                ins.sync_info = mybir.SyncInfo(on_wait=keep, on_update=si.on_update)
                for w in extra:
                    _WS_COUNT[0] += 1
                    nop = mybir.InstNoOp(name=f"I-ws{_WS_COUNT[0]}", ins=[], outs=[])
                    nop.engine = ins.engine
                    nop.sync_info = mybir.SyncInfo(on_wait=[w], on_update=[])
                    insts.insert(i, nop)
                    i += 1
                i += 1



B, N, D = 16384, 50, 16
NCORES = 8
BC = B // NCORES            # 2048 real samples per core
K = D + 1                   # 17 rows per sample
KT = 3 * K                  # 51 rows per trio
MT = 3 * D                  # 48 output rows per trio
SLT = N + KT + MT           # 149 cols per trio slot
BCP = 2112                  # padded samples per core (64 junk, 3.1%)
SLOTS = BCP // 6            # 352 tile-slots (2 trios each) per core
QUADS = SLOTS               # alias for the test harness
G = 32                      # slots per DMA batch -> 11 batches
GS = 8                      # slots per PSUM sub-batch

F32 = mybir.dt.float32
F16 = mybir.dt.float16


def build_nc(nq=SLOTS, g=G, gs=GS, dt=F16, sim_mode=False):
    """DRAM (per core), batch-major:
      ch [nbatch, 2*KT, g*SLT] : row KT*j+r, col qq*SLT+c = trio (slot qq, half j)
      yh [nbatch, 2*MT, g*N]   : row MT*j+e
    """
    assert nq % g == 0 and g % gs == 0
    nbatch = nq // g
    nsub = g // gs
    cf = SLT * g
    xf = N * g
    sf = N * gs
    cfp = cf + 8
    xfp = xf + 8

    nc = bass.Bass(
        "TRN2",
        target_bir_lowering=False,
        debug=False,
        detect_race_conditions=False,
    )
    ch = nc.dram_tensor("ch", [nbatch, 2 * KT, cf], dt, kind="ExternalInput")
    yh = nc.dram_tensor("yh", [nbatch, 2 * MT, xf], dt, kind="ExternalOutput")

    relu = mybir.ActivationFunctionType.Relu

    with tile.TileContext(nc) as tc, ExitStack() as ctx:
        cpool = ctx.enter_context(tc.tile_pool(name="cpool", bufs=4))
        hpool = ctx.enter_context(tc.tile_pool(name="hpool", bufs=4))
        ypool = ctx.enter_context(tc.tile_pool(name="ypool", bufs=3))
        pspool = ctx.enter_context(tc.tile_pool(name="ps", bufs=4, space="PSUM"))

        prev_sp = [None]

        def sp_chain(inst):
            if prev_sp[0] is not None:
                add_dep_helper(inst.ins, prev_sp[0].ins, sync=False,
                               reason="SP issue order")
            prev_sp[0] = inst
            return inst

        cts = {}

        def emit_in_dma(bi):
            ct = cpool.tile([128, cfp], dt, name="ct")
            cts[bi] = ct
            nsplit = 4 if bi == 0 else 1
            csz = cf // nsplit
            for j in range(2):
                for sp in range(nsplit):
                    eng = nc.sync if bi == 0 and sp == 0 else nc.gpsimd
                    eng.dma_start(
                        bass.AP(ct.tensor, 64 * j * cfp + sp * csz,
                                [[cfp, KT], [1, csz]]),
                        bass.AP(ch, (bi * 2 + j) * KT * cf + sp * csz,
                                [[cf, KT], [1, csz]]),
                    )

        for pb in range(min(3, nbatch)):
            emit_in_dma(pb)

        subs = [(bi, s) for bi in range(nbatch) for s in range(nsub)]
        state = {}
        yts = {}

        def emit_l1(ss):
            bi, s = subs[ss]
            if s == 0:
                if bi + 3 < nbatch:
                    emit_in_dma(bi + 3)
                yts[bi] = ypool.tile([128, xfp], dt, name="yt")
            ct = cts[bi]
            pw = 512 + sf
            ps = pspool.tile([128, pw], F32, name="ps")
            if sim_mode:
                nc.vector.memset(ps[:, :], 0.0)
            for q in range(gs):
                qq = s * gs + q
                for j in range(2):
                    nc.tensor.matmul(
                        bass.AP(ps.tensor, 64 * j * pw + q * N, [[pw, KT], [1, N]]),
                        bass.AP(ct.tensor, 64 * j * cfp + qq * SLT + N, [[cfp, KT], [1, KT]]),
                        bass.AP(ct.tensor, 64 * j * cfp + qq * SLT, [[cfp, KT], [1, N]]),
                        start=True,
                        stop=True,
                        tile_position=(64 * j, 64 * j),
                    )
            ht = hpool.tile([128, sf], dt, name="ht")
            nc.scalar.activation(
                bass.AP(ht.tensor, 0, [[sf, 128], [1, sf]]),
                bass.AP(ps.tensor, 0, [[pw, 128], [1, sf]]),
                relu,
            )
            state[ss] = (ct, ht, ps)

        def emit_l2(ss):
            bi, s = subs[ss]
            ct, ht, ps = state.pop(ss)
            yt = yts[bi]
            pw = 512 + sf
            for q in range(gs):
                qq = s * gs + q
                for j in range(2):
                    nc.tensor.matmul(
                        bass.AP(ps.tensor, 64 * j * pw + 512 + q * N, [[pw, MT], [1, N]]),
                        bass.AP(ct.tensor, 64 * j * cfp + qq * SLT + N + KT, [[cfp, KT], [1, MT]]),
                        bass.AP(ht.tensor, 64 * j * sf + q * N, [[sf, KT], [1, N]]),
                        start=True,
                        stop=True,
                        tile_position=(64 * j, 64 * j),
                    )
            nc.vector.tensor_scalar_max(
                bass.AP(yt.tensor, s * sf, [[xfp, 128], [1, sf]]),
                bass.AP(ps.tensor, 512, [[pw, 128], [1, sf]]),
                0.0,
            )
            if s == nsub - 1:
                for j in range(2):
                    sp_chain(nc.sync.dma_start(
                        bass.AP(yh, (bi * 2 + j) * MT * xf, [[xf, MT], [1, xf]]),
                        bass.AP(yt.tensor, 64 * j * xfp, [[xfp, MT], [1, xf]]),
                    ))
                cts.pop(bi)

        SKEW = 2
        for idx in range(len(subs) + SKEW):
            if idx < len(subs):
                emit_l1(idx)
            if idx >= SKEW:
                emit_l2(idx - SKEW)

    _strip_covered_waits(nc)
    _split_excess_waits(nc)
    return nc


def pack_inputs(user_emb, item_emb, nq=SLOTS, g=G, dt=np.float16):
    ncores = NCORES
    nbatch = nq // g
    x = np.ascontiguousarray(user_emb, dtype=np.float32)
    ie = np.ascontiguousarray(item_emb, dtype=np.float32)

    # per-sample building blocks (real samples only)
    xt = np.zeros((B, K, N), dtype=np.float32)
    xt[:, :D] = x.transpose(0, 2, 1)
    xt[:, D] = 1.0
    w0 = np.zeros((B, K, K), dtype=np.float32)
    w0[:, :D, :D] = ie[:, : D * D].reshape(B, D, D)
    w0[:, D, :D] = ie[:, D * D : D * D + D]
    w0[:, D, D] = 1.0
    off = D * (D + 1)
    w1 = np.zeros((B, K, D), dtype=np.float32)
    w1[:, :D] = ie[:, off : off + D * D].reshape(B, D, D)
    w1[:, D] = ie[:, off + D * D : off + D * D + D]

    # pad per core: 2048 real -> 2112 (junk zeros at the tail of each core)
    def pad_core(a):
        a = a.reshape(ncores, BC, *a.shape[1:])
        pad = np.zeros((ncores, BCP - BC, *a.shape[2:]), dtype=a.dtype)
        return np.concatenate([a, pad], axis=1)

    xtp, w0p, w1p = pad_core(xt), pad_core(w0), pad_core(w1)

    ntrio = ncores * (BCP // 3)
    comb = np.zeros((ntrio, KT, SLT), dtype=np.float32)
    xt3 = xtp.reshape(ntrio, 3, K, N)
    w03 = w0p.reshape(ntrio, 3, K, K)
    w13 = w1p.reshape(ntrio, 3, K, D)
    for i in range(3):
        rows = slice(K * i, K * i + K)
        comb[:, rows, :N] = xt3[:, i]
        comb[:, rows, N + K * i : N + K * i + K] = w03[:, i]
        comb[:, rows, N + KT + D * i : N + KT + D * i + D] = w13[:, i]

    # trio t (within core) -> slot qq = t // 2, half j = t % 2
    chs = (
        comb.reshape(ncores, nbatch, g, 2, KT, SLT)
        .transpose(0, 1, 3, 4, 2, 5)        # c, bi, j, row, qq, col
        .astype(dt, copy=False)
    )
    return [
        {"ch": np.ascontiguousarray(chs[c]).reshape(nbatch, 2 * KT, g * SLT)}
        for c in range(ncores)
    ]


def unpack_output(results, nq=SLOTS, g=G):
    nbatch = nq // g
    yh = np.stack([r["yh"] for r in results])
    y = (
        yh.reshape(NCORES, nbatch, 2, 3, D, g, N)   # c, bi, j, i, e, qq, n
        .transpose(0, 1, 5, 2, 3, 6, 4)             # c, bi, qq, j, i, n, e
    )
    y = y.reshape(NCORES, BCP, N, D)[:, :BC]        # drop junk tail
    return np.ascontiguousarray(y.reshape(B, N, D), dtype=np.float32)


_NC_CACHE = {}


def _get_nc(key=(SLOTS, G)):
    if key not in _NC_CACHE:
        nq, g = key
        _NC_CACHE[key] = build_nc(nq=nq, g=g)
    return _NC_CACHE[key]


def kernel(user_emb, item_emb):
    nc = _get_nc()
    in_maps = pack_inputs(user_emb, item_emb)
    res = run_bass_kernel_spmd(nc, in_maps, core_ids=list(range(NCORES)))
    return unpack_output(res.results)


# revision 24
# speedup vs baseline: 1.0149x; 1.0149x over previous
"""CAN per-sample 2-layer MLP kernel for Trainium2 — P=3 trio variant.

Like the pair kernel, but THREE samples share each matmul:
  * Trio slot: samples at rows {0,17,34} of a 51-row block; two trios per
    128-partition tile at 64-row strides, matmuls at tile_position
    (64j, 64j) so consecutive matmuls alternate row groups and LDWEIGHTS
    pulls ahead (51 cols ~ 21ns @2.4GHz hides under the 50-row stream).
  * L1: 51x51 block-diag Wt0; L2: 51x48 stacked Wt1; out 48 rows/trio.
  * Per core: 2048 samples padded to 2112 = 352 tile-slots (6 samples
    each) = 11 batches x 32 slots; PSUM sub-batch gs=8 slots; 1408
    matmuls/core vs 2048 for pairs.
  * Everything else mirrors kernel.py: fp16 wire, fp32 PSUM, one
    two-bank psum tile per sub-batch, full-width relu1 on ACT / relu2 on
    DVE, SWDGE input DMAs 3 batches ahead, SKEW=2 pipeline, wait
    stripping post-passes.
"""

from contextlib import ExitStack

import numpy as np

import concourse.bass as bass
import concourse.mybir as mybir
from concourse import tile
from concourse.bass_utils import run_bass_kernel_spmd
from concourse.tile_rust import add_dep_helper

def _strip_covered_waits(nc):
    """Remove, from DMACopy instructions, semaphore waits already guaranteed
    by an earlier instruction on the same engine queue. Coverage is killed
    for a sem from the point of any non-increment update (barrier resets)."""
    for fn in nc.m.functions:
        for blk in fn.blocks:
            seen = {}
            for ins in blk.instructions:
                si = ins.sync_info
                if si is None:
                    continue
                eng = ins.engine
                # In-order sequencers: a sem-ge wait satisfied for an earlier
                # instruction on this queue still holds for later ones, so
                # repeats are dead weight. On PE they also fence the 64-deep
                # reorder window (LDWEIGHTS pull-ahead), serializing
                # ldweights with the previous matmul.
                strippable = type(ins).__name__ in ("InstDMACopy", "InstMatmult")
                kept = []
                changed = False
                for w in si.on_wait:
                    if (
                        strippable
                        and w.wait_mode == "sem-ge-imm"
                        and w.wait_reg is None
                        and seen.get((eng, w.id), -1) >= w.wait_value
                    ):
                        changed = True
                        continue
                    kept.append(w)
                for w in kept:
                    if w.wait_mode == "sem-ge-imm" and w.wait_reg is None:
                        key = (eng, w.id)
                        if seen.get(key, -1) < w.wait_value:
                            seen[key] = w.wait_value
                for u in si.on_update:
                    if u.update_mode != "sem-add-imm" or (
                        u.update_value is not None and u.update_value < 0
                    ):
                        for key in [k for k in seen if k[1] == u.id]:
                            del seen[key]
                if changed:
                    ins.sync_info = mybir.SyncInfo(
                        on_wait=kept, on_update=si.on_update
                    )


_WS_COUNT = [0]


def _split_excess_waits(nc, cap=1):
    """Move excess inline waits onto NoOps inserted immediately before, on
    the same engine queue - semantically identical (sequencers execute
    waits in order)."""
    for fn in nc.m.functions:
        for blk in fn.blocks:
            insts = blk.instructions
            i = 0
            while i < len(insts):
                ins = insts[i]
                si = ins.sync_info
                if si is None or len(si.on_wait) <= cap:
                    i += 1
                    continue
                waits = list(si.on_wait)
                keep, extra = waits[-cap:], waits[:-cap]
                ins.sync_info = mybir.SyncInfo(on_wait=keep, on_update=si.on_update)
                for w in extra:
                    _WS_COUNT[0] += 1
                    nop = mybir.InstNoOp(name=f"I-ws{_WS_COUNT[0]}", ins=[], outs=[])
                    nop.engine = ins.engine
                    nop.sync_info = mybir.SyncInfo(on_wait=[w], on_update=[])
                    insts.insert(i, nop)
                    i += 1
                i += 1



B, N, D = 16384, 50, 16
NCORES = 8
BC = B // NCORES            # 2048 real samples per core
K = D + 1                   # 17 rows per sample
KT = 3 * K                  # 51 rows per trio
MT = 3 * D                  # 48 output rows per trio
SLT = N + KT + MT           # 149 cols per trio slot
BCP = 2112                  # padded samples per core (64 junk, 3.1%)
SLOTS = BCP // 6            # 352 tile-slots (2 trios each) per core
QUADS = SLOTS               # alias for the test harness
G = 32                      # slots per DMA batch -> 11 batches
GS = 8                      # slots per PSUM sub-batch

F32 = mybir.dt.float32
F16 = mybir.dt.float16


def build_nc(nq=SLOTS, g=G, gs=GS, dt=F16, sim_mode=False):
    """DRAM (per core), batch-major:
      ch [nbatch, 2*KT, g*SLT] : row KT*j+r, col qq*SLT+c = trio (slot qq, half j)
      yh [nbatch, 2*MT, g*N]   : row MT*j+e
    """
    assert nq % g == 0 and g % gs == 0
    nbatch = nq // g
    nsub = g // gs
    cf = SLT * g
    xf = N * g
    sf = N * gs
    cfp = cf + 8
    xfp = xf + 8

    nc = bass.Bass(
        "TRN2",
        target_bir_lowering=False,
        debug=False,
        detect_race_conditions=False,
    )
    ch = nc.dram_tensor("ch", [nbatch, 2 * KT, cf], dt, kind="ExternalInput")
    yh = nc.dram_tensor("yh", [nbatch, 2 * MT, xf], dt, kind="ExternalOutput")

    relu = mybir.ActivationFunctionType.Relu

    with tile.TileContext(nc) as tc, ExitStack() as ctx:
        cpool = ctx.enter_context(tc.tile_pool(name="cpool", bufs=4))
        hpool = ctx.enter_context(tc.tile_pool(name="hpool", bufs=4))
        ypool = ctx.enter_context(tc.tile_pool(name="ypool", bufs=3))
        pspool = ctx.enter_context(tc.tile_pool(name="ps", bufs=4, space="PSUM"))

        prev_sp = [None]

        def sp_chain(inst):
            if prev_sp[0] is not None:
                add_dep_helper(inst.ins, prev_sp[0].ins, sync=False,
                               reason="SP issue order")
            prev_sp[0] = inst
            return inst

        cts = {}

        def in_piece(bi, j, lo, hi):
            nc.gpsimd.dma_start(
                bass.AP(cts[bi].tensor, 64 * j * cfp + lo,
                        [[cfp, KT], [1, hi - lo]]),
                bass.AP(ch, (bi * 2 + j) * KT * cf + lo,
                        [[cf, KT], [1, hi - lo]]),
            )

        def emit_in_dma(bi):
            cts[bi] = cpool.tile([128, cfp], dt, name="ct")
            in_piece(bi, 0, 0, cf)
            in_piece(bi, 1, 0, cf)

        # Startup: the GpSimd SWDGE sequencer generates ~1 DMA/us, so issue
        # in NEED order: sub-batch 0's two halves first (sprayed across all
        # 16 engines they land ~0.3us after gen), then the batch-0
        # remainder, then batches 1-2. A Sync-queue piece is WORSE here:
        # HWDGE pins the load to one SDMA engine (~5.4us for 122KB).
        q4 = cf // 4
        cts[0] = cpool.tile([128, cfp], dt, name="ct")
        in_piece(0, 0, 0, q4)
        in_piece(0, 1, 0, q4)
        in_piece(0, 0, q4, cf)
        in_piece(0, 1, q4, cf)
        cts[1] = cpool.tile([128, cfp], dt, name="ct")
        in_piece(1, 0, 0, cf)
        in_piece(1, 1, 0, cf)
        emit_in_dma(2)

        subs = [(bi, s) for bi in range(nbatch) for s in range(nsub)]
        state = {}
        yts = {}

        def emit_l1(ss):
            bi, s = subs[ss]
            if s == 0:
                if bi + 3 < nbatch:
                    emit_in_dma(bi + 3)
                yts[bi] = ypool.tile([128, xfp], dt, name="yt")
            ct = cts[bi]
            pw = 512 + sf
            ps = pspool.tile([128, pw], F32, name="ps")
            if sim_mode:
                nc.vector.memset(ps[:, :], 0.0)
            for q in range(gs):
                qq = s * gs + q
                for j in range(2):
                    nc.tensor.matmul(
                        bass.AP(ps.tensor, 64 * j * pw + q * N, [[pw, KT], [1, N]]),
                        bass.AP(ct.tensor, 64 * j * cfp + qq * SLT + N, [[cfp, KT], [1, KT]]),
                        bass.AP(ct.tensor, 64 * j * cfp + qq * SLT, [[cfp, KT], [1, N]]),
                        start=True,
                        stop=True,
                        tile_position=(64 * j, 64 * j),
                    )
            ht = hpool.tile([128, sf], dt, name="ht")
            nc.scalar.activation(
                bass.AP(ht.tensor, 0, [[sf, 128], [1, sf]]),
                bass.AP(ps.tensor, 0, [[pw, 128], [1, sf]]),
                relu,
            )
            state[ss] = (ct, ht, ps)

        def emit_l2(ss):
            bi, s = subs[ss]
            ct, ht, ps = state.pop(ss)
            yt = yts[bi]
            pw = 512 + sf
            for q in range(gs):
                qq = s * gs + q
                for j in range(2):
                    nc.tensor.matmul(
                        bass.AP(ps.tensor, 64 * j * pw + 512 + q * N, [[pw, MT], [1, N]]),
                        bass.AP(ct.tensor, 64 * j * cfp + qq * SLT + N + KT, [[cfp, KT], [1, MT]]),
                        bass.AP(ht.tensor, 64 * j * sf + q * N, [[sf, KT], [1, N]]),
                        start=True,
                        stop=True,
                        tile_position=(64 * j, 64 * j),
                    )
            nc.vector.tensor_scalar_max(
                bass.AP(yt.tensor, s * sf, [[xfp, 128], [1, sf]]),
                bass.AP(ps.tensor, 512, [[pw, 128], [1, sf]]),
                0.0,
            )
            if s == nsub - 1:
                for j in range(2):
                    sp_chain(nc.sync.dma_start(
                        bass.AP(yh, (bi * 2 + j) * MT * xf, [[xf, MT], [1, xf]]),
                        bass.AP(yt.tensor, 64 * j * xfp, [[xfp, MT], [1, xf]]),
                    ))
                cts.pop(bi)

        SKEW = 2
        for idx in range(len(subs) + SKEW):
            if idx < len(subs):
                emit_l1(idx)
            if idx >= SKEW:
                emit_l2(idx - SKEW)

    _strip_covered_waits(nc)
    _split_excess_waits(nc)
    return nc


def pack_inputs(user_emb, item_emb, nq=SLOTS, g=G, dt=np.float16):
    ncores = NCORES
    nbatch = nq // g
    x = np.ascontiguousarray(user_emb, dtype=np.float32)
    ie = np.ascontiguousarray(item_emb, dtype=np.float32)

    # per-sample building blocks (real samples only)
    xt = np.zeros((B, K, N), dtype=np.float32)
    xt[:, :D] = x.transpose(0, 2, 1)
    xt[:, D] = 1.0
    w0 = np.zeros((B, K, K), dtype=np.float32)
    w0[:, :D, :D] = ie[:, : D * D].reshape(B, D, D)
    w0[:, D, :D] = ie[:, D * D : D * D + D]
    w0[:, D, D] = 1.0
    off = D * (D + 1)
    w1 = np.zeros((B, K, D), dtype=np.float32)
    w1[:, :D] = ie[:, off : off + D * D].reshape(B, D, D)
    w1[:, D] = ie[:, off + D * D : off + D * D + D]

    # pad per core: 2048 real -> 2112 (junk zeros at the tail of each core)
    def pad_core(a):
        a = a.reshape(ncores, BC, *a.shape[1:])
        pad = np.zeros((ncores, BCP - BC, *a.shape[2:]), dtype=a.dtype)
        return np.concatenate([a, pad], axis=1)

    xtp, w0p, w1p = pad_core(xt), pad_core(w0), pad_core(w1)

    ntrio = ncores * (BCP // 3)
    comb = np.zeros((ntrio, KT, SLT), dtype=np.float32)
    xt3 = xtp.reshape(ntrio, 3, K, N)
    w03 = w0p.reshape(ntrio, 3, K, K)
    w13 = w1p.reshape(ntrio, 3, K, D)
    for i in range(3):
        rows = slice(K * i, K * i + K)
        comb[:, rows, :N] = xt3[:, i]
        comb[:, rows, N + K * i : N + K * i + K] = w03[:, i]
        comb[:, rows, N + KT + D * i : N + KT + D * i + D] = w13[:, i]

    # trio t (within core) -> slot qq = t // 2, half j = t % 2
    chs = (
        comb.reshape(ncores, nbatch, g, 2, KT, SLT)
        .transpose(0, 1, 3, 4, 2, 5)        # c, bi, j, row, qq, col
        .astype(dt, copy=False)
    )
    return [
        {"ch": np.ascontiguousarray(chs[c]).reshape(nbatch, 2 * KT, g * SLT)}
        for c in range(ncores)
    ]


def unpack_output(results, nq=SLOTS, g=G):
    nbatch = nq // g
    yh = np.stack([r["yh"] for r in results])
    y = (
        yh.reshape(NCORES, nbatch, 2, 3, D, g, N)   # c, bi, j, i, e, qq, n
        .transpose(0, 1, 5, 2, 3, 6, 4)             # c, bi, qq, j, i, n, e
    )
    y = y.reshape(NCORES, BCP, N, D)[:, :BC]        # drop junk tail
    return np.ascontiguousarray(y.reshape(B, N, D), dtype=np.float32)


_NC_CACHE = {}


def _get_nc(key=(SLOTS, G)):
    if key not in _NC_CACHE:
        nq, g = key
        _NC_CACHE[key] = build_nc(nq=nq, g=g)
    return _NC_CACHE[key]


def kernel(user_emb, item_emb):
    nc = _get_nc()
    in_maps = pack_inputs(user_emb, item_emb)
    res = run_bass_kernel_spmd(nc, in_maps, core_ids=list(range(NCORES)))
    return unpack_output(res.results)


# revision 25
# speedup vs baseline: 1.0282x; 1.0131x over previous
"""CAN per-sample 2-layer MLP kernel for Trainium2 — P=3 trio variant.

Like the pair kernel, but THREE samples share each matmul:
  * Trio slot: samples at rows {0,17,34} of a 51-row block; two trios per
    128-partition tile at 64-row strides, matmuls at tile_position
    (64j, 64j) so consecutive matmuls alternate row groups and LDWEIGHTS
    pulls ahead (51 cols ~ 21ns @2.4GHz hides under the 50-row stream).
  * L1: 51x51 block-diag Wt0; L2: 51x48 stacked Wt1; out 48 rows/trio.
  * Per core: 2048 samples padded to 2112 = 352 tile-slots (6 samples
    each) = 11 batches x 32 slots; PSUM sub-batch gs=8 slots; 1408
    matmuls/core vs 2048 for pairs.
  * Everything else mirrors kernel.py: fp16 wire, fp32 PSUM, one
    two-bank psum tile per sub-batch, full-width relu1 on ACT / relu2 on
    DVE, SWDGE input DMAs 3 batches ahead, SKEW=2 pipeline, wait
    stripping post-passes.
"""

from contextlib import ExitStack

import numpy as np

import concourse.bass as bass
import concourse.mybir as mybir
from concourse import tile
from concourse.bass_utils import run_bass_kernel_spmd
from concourse.tile_rust import add_dep_helper

def _strip_covered_waits(nc):
    """Remove, from DMACopy instructions, semaphore waits already guaranteed
    by an earlier instruction on the same engine queue. Coverage is killed
    for a sem from the point of any non-increment update (barrier resets)."""
    for fn in nc.m.functions:
        for blk in fn.blocks:
            seen = {}
            for ins in blk.instructions:
                si = ins.sync_info
                if si is None:
                    continue
                eng = ins.engine
                # In-order sequencers: a sem-ge wait satisfied for an earlier
                # instruction on this queue still holds for later ones, so
                # repeats are dead weight. On PE they also fence the 64-deep
                # reorder window (LDWEIGHTS pull-ahead), serializing
                # ldweights with the previous matmul.
                strippable = type(ins).__name__ in ("InstDMACopy", "InstMatmult")
                kept = []
                changed = False
                for w in si.on_wait:
                    if (
                        strippable
                        and w.wait_mode == "sem-ge-imm"
                        and w.wait_reg is None
                        and seen.get((eng, w.id), -1) >= w.wait_value
                    ):
                        changed = True
                        continue
                    kept.append(w)
                for w in kept:
                    if w.wait_mode == "sem-ge-imm" and w.wait_reg is None:
                        key = (eng, w.id)
                        if seen.get(key, -1) < w.wait_value:
                            seen[key] = w.wait_value
                for u in si.on_update:
                    if u.update_mode != "sem-add-imm" or (
                        u.update_value is not None and u.update_value < 0
                    ):
                        for key in [k for k in seen if k[1] == u.id]:
                            del seen[key]
                if changed:
                    ins.sync_info = mybir.SyncInfo(
                        on_wait=kept, on_update=si.on_update
                    )


_WS_COUNT = [0]


def _split_excess_waits(nc, cap=1):
    """Move excess inline waits onto NoOps inserted immediately before, on
    the same engine queue - semantically identical (sequencers execute
    waits in order)."""
    for fn in nc.m.functions:
        for blk in fn.blocks:
            insts = blk.instructions
            i = 0
            while i < len(insts):
                ins = insts[i]
                si = ins.sync_info
                if si is None or len(si.on_wait) <= cap:
                    i += 1
                    continue
                waits = list(si.on_wait)
                keep, extra = waits[-cap:], waits[:-cap]
                ins.sync_info = mybir.SyncInfo(on_wait=keep, on_update=si.on_update)
                for w in extra:
                    _WS_COUNT[0] += 1
                    nop = mybir.InstNoOp(name=f"I-ws{_WS_COUNT[0]}", ins=[], outs=[])
                    nop.engine = ins.engine
                    nop.sync_info = mybir.SyncInfo(on_wait=[w], on_update=[])
                    insts.insert(i, nop)
                    i += 1
                i += 1



B, N, D = 16384, 50, 16
NCORES = 8
BC = B // NCORES            # 2048 real samples per core
K = D + 1                   # 17 rows per sample
KT = 3 * K                  # 51 rows per trio
MT = 3 * D                  # 48 output rows per trio
SLT = N + KT + MT           # 149 cols per trio slot
BCP = 2112                  # padded samples per core (64 junk, 3.1%)
SLOTS = BCP // 6            # 352 tile-slots (2 trios each) per core
QUADS = SLOTS               # alias for the test harness
G = 32                      # slots per DMA batch -> 11 batches
GS = 8                      # slots per PSUM sub-batch

F32 = mybir.dt.float32
F16 = mybir.dt.float16


def build_nc(nq=SLOTS, g=G, gs=GS, dt=F16, sim_mode=False):
    """DRAM (per core), batch-major:
      ch [nbatch, 2*KT, g*SLT] : row KT*j+r, col qq*SLT+c = trio (slot qq, half j)
      yh [nbatch, 2*MT, g*N]   : row MT*j+e
    """
    assert nq % g == 0 and g % gs == 0
    nbatch = nq // g
    nsub = g // gs
    cf = SLT * g
    xf = N * g
    sf = N * gs
    cfp = cf + 8
    xfp = xf + 8

    nc = bass.Bass(
        "TRN2",
        target_bir_lowering=False,
        debug=False,
        detect_race_conditions=False,
    )
    ch = nc.dram_tensor("ch", [nbatch, 2 * KT, cf], dt, kind="ExternalInput")
    yh = nc.dram_tensor("yh", [nbatch, 2 * MT, xf], dt, kind="ExternalOutput")

    relu = mybir.ActivationFunctionType.Relu

    with tile.TileContext(nc) as tc, ExitStack() as ctx:
        cpool = ctx.enter_context(tc.tile_pool(name="cpool", bufs=4))
        hpool = ctx.enter_context(tc.tile_pool(name="hpool", bufs=4))
        ypool = ctx.enter_context(tc.tile_pool(name="ypool", bufs=3))
        pspool = ctx.enter_context(tc.tile_pool(name="ps", bufs=4, space="PSUM"))

        prev_sp = [None]

        def sp_chain(inst):
            if prev_sp[0] is not None:
                add_dep_helper(inst.ins, prev_sp[0].ins, sync=False,
                               reason="SP issue order")
            prev_sp[0] = inst
            return inst

        cts = {}

        def in_piece(bi, j, lo, hi):
            nc.gpsimd.dma_start(
                bass.AP(cts[bi].tensor, 64 * j * cfp + lo,
                        [[cfp, KT], [1, hi - lo]]),
                bass.AP(ch, (bi * 2 + j) * KT * cf + lo,
                        [[cf, KT], [1, hi - lo]]),
            )

        def emit_in_dma(bi):
            cts[bi] = cpool.tile([128, cfp], dt, name="ct")
            in_piece(bi, 0, 0, cf)
            in_piece(bi, 1, 0, cf)

        # Startup: the GpSimd SWDGE sequencer generates ~1 DMA/us, so issue
        # in NEED order: sub-batch 0's two halves first (sprayed across all
        # 16 engines they land ~0.3us after gen), then the batch-0
        # remainder, then batches 1-2. A Sync-queue piece is WORSE here:
        # HWDGE pins the load to one SDMA engine (~5.4us for 122KB).
        q4 = cf // 4
        cts[0] = cpool.tile([128, cfp], dt, name="ct")
        in_piece(0, 0, 0, q4)
        in_piece(0, 1, 0, q4)
        in_piece(0, 0, q4, cf)
        in_piece(0, 1, q4, cf)
        cts[1] = cpool.tile([128, cfp], dt, name="ct")
        in_piece(1, 0, 0, cf)
        in_piece(1, 1, 0, cf)
        emit_in_dma(2)

        subs = [(bi, s) for bi in range(nbatch) for s in range(nsub)]
        state = {}
        yts = {}

        def emit_l1(ss):
            bi, s = subs[ss]
            if s == 0:
                if bi + 3 < nbatch:
                    emit_in_dma(bi + 3)
                yts[bi] = ypool.tile([128, xfp], dt, name="yt")
            ct = cts[bi]
            # Separate L1/L2 psum tiles: a merged two-bank tile is freed
            # only after BOTH relus read it, so L1 heads ended up waiting
            # on the relu2 (DVE) chain tail. Alternating per-layer tiles
            # make L1 wait only on the much earlier relu1.
            ps = pspool.tile([128, sf], F32, name="ps1")
            if sim_mode:
                nc.vector.memset(ps[:, :], 0.0)
            for q in range(gs):
                qq = s * gs + q
                for j in range(2):
                    nc.tensor.matmul(
                        bass.AP(ps.tensor, 64 * j * sf + q * N, [[sf, KT], [1, N]]),
                        bass.AP(ct.tensor, 64 * j * cfp + qq * SLT + N, [[cfp, KT], [1, KT]]),
                        bass.AP(ct.tensor, 64 * j * cfp + qq * SLT, [[cfp, KT], [1, N]]),
                        start=True,
                        stop=True,
                        tile_position=(64 * j, 64 * j),
                    )
            ht = hpool.tile([128, sf], dt, name="ht")
            nc.scalar.activation(
                bass.AP(ht.tensor, 0, [[sf, 128], [1, sf]]),
                bass.AP(ps.tensor, 0, [[sf, 128], [1, sf]]),
                relu,
            )
            state[ss] = (ct, ht, ps)

        def emit_l2(ss):
            bi, s = subs[ss]
            ct, ht, ps = state.pop(ss)
            yt = yts[bi]
            ps2 = pspool.tile([128, sf], F32, name="ps2")
            if sim_mode:
                nc.vector.memset(ps2[:, :], 0.0)
            for q in range(gs):
                qq = s * gs + q
                for j in range(2):
                    nc.tensor.matmul(
                        bass.AP(ps2.tensor, 64 * j * sf + q * N, [[sf, MT], [1, N]]),
                        bass.AP(ct.tensor, 64 * j * cfp + qq * SLT + N + KT, [[cfp, KT], [1, MT]]),
                        bass.AP(ht.tensor, 64 * j * sf + q * N, [[sf, KT], [1, N]]),
                        start=True,
                        stop=True,
                        tile_position=(64 * j, 64 * j),
                    )
            nc.vector.tensor_scalar_max(
                bass.AP(yt.tensor, s * sf, [[xfp, 128], [1, sf]]),
                bass.AP(ps2.tensor, 0, [[sf, 128], [1, sf]]),
                0.0,
            )
            if s == nsub - 1:
                for j in range(2):
                    sp_chain(nc.sync.dma_start(
                        bass.AP(yh, (bi * 2 + j) * MT * xf, [[xf, MT], [1, xf]]),
                        bass.AP(yt.tensor, 64 * j * xfp, [[xfp, MT], [1, xf]]),
                    ))
                cts.pop(bi)

        SKEW = 2
        for idx in range(len(subs) + SKEW):
            if idx < len(subs):
                emit_l1(idx)
            if idx >= SKEW:
                emit_l2(idx - SKEW)

    _strip_covered_waits(nc)
    _split_excess_waits(nc)
    return nc


def pack_inputs(user_emb, item_emb, nq=SLOTS, g=G, dt=np.float16):
    ncores = NCORES
    nbatch = nq // g
    x = np.ascontiguousarray(user_emb, dtype=np.float32)
    ie = np.ascontiguousarray(item_emb, dtype=np.float32)

    # per-sample building blocks (real samples only)
    xt = np.zeros((B, K, N), dtype=np.float32)
    xt[:, :D] = x.transpose(0, 2, 1)
    xt[:, D] = 1.0
    w0 = np.zeros((B, K, K), dtype=np.float32)
    w0[:, :D, :D] = ie[:, : D * D].reshape(B, D, D)
    w0[:, D, :D] = ie[:, D * D : D * D + D]
    w0[:, D, D] = 1.0
    off = D * (D + 1)
    w1 = np.zeros((B, K, D), dtype=np.float32)
    w1[:, :D] = ie[:, off : off + D * D].reshape(B, D, D)
    w1[:, D] = ie[:, off + D * D : off + D * D + D]

    # pad per core: 2048 real -> 2112 (junk zeros at the tail of each core)
    def pad_core(a):
        a = a.reshape(ncores, BC, *a.shape[1:])
        pad = np.zeros((ncores, BCP - BC, *a.shape[2:]), dtype=a.dtype)
        return np.concatenate([a, pad], axis=1)

    xtp, w0p, w1p = pad_core(xt), pad_core(w0), pad_core(w1)

    ntrio = ncores * (BCP // 3)
    comb = np.zeros((ntrio, KT, SLT), dtype=np.float32)
    xt3 = xtp.reshape(ntrio, 3, K, N)
    w03 = w0p.reshape(ntrio, 3, K, K)
    w13 = w1p.reshape(ntrio, 3, K, D)
    for i in range(3):
        rows = slice(K * i, K * i + K)
        comb[:, rows, :N] = xt3[:, i]
        comb[:, rows, N + K * i : N + K * i + K] = w03[:, i]
        comb[:, rows, N + KT + D * i : N + KT + D * i + D] = w13[:, i]

    # trio t (within core) -> slot qq = t // 2, half j = t % 2
    chs = (
        comb.reshape(ncores, nbatch, g, 2, KT, SLT)
        .transpose(0, 1, 3, 4, 2, 5)        # c, bi, j, row, qq, col
        .astype(dt, copy=False)
    )
    return [
        {"ch": np.ascontiguousarray(chs[c]).reshape(nbatch, 2 * KT, g * SLT)}
        for c in range(ncores)
    ]


def unpack_output(results, nq=SLOTS, g=G):
    nbatch = nq // g
    yh = np.stack([r["yh"] for r in results])
    y = (
        yh.reshape(NCORES, nbatch, 2, 3, D, g, N)   # c, bi, j, i, e, qq, n
        .transpose(0, 1, 5, 2, 3, 6, 4)             # c, bi, qq, j, i, n, e
    )
    y = y.reshape(NCORES, BCP, N, D)[:, :BC]        # drop junk tail
    return np.ascontiguousarray(y.reshape(B, N, D), dtype=np.float32)


_NC_CACHE = {}


def _get_nc(key=(SLOTS, G)):
    if key not in _NC_CACHE:
        nq, g = key
        _NC_CACHE[key] = build_nc(nq=nq, g=g)
    return _NC_CACHE[key]


def kernel(user_emb, item_emb):
    nc = _get_nc()
    in_maps = pack_inputs(user_emb, item_emb)
    res = run_bass_kernel_spmd(nc, in_maps, core_ids=list(range(NCORES)))
    return unpack_output(res.results)
